# revision 32
# baseline (speedup 1.0000x reference)
"""Block-causal (frame-windowed) attention layer for Trainium2, 8-core SPMD.

Reference computation (B=4, T=2048, C=512, H=8, Dh=64, NPATCH=256):
  LayerNorm(x) -> qkv = xn @ w_qkv -> per-head attention with mask
  frame(i) >= frame(j), frame = idx // 256 -> out @ w_out + b_out

Sharding: core c handles batch c//2 and heads (c%2)*4 .. (c%2)*4+3.
Each core computes a partial y (its heads' contribution to out @ w_out);
the host sums the two partials per batch and adds b_out.

Design (v2):
 - host sends x pre-transposed (xT [C, T] fp16), so no PE transposes.
 - LayerNorm via matmul: col-sums of xT and xT^2 with a ones weight give
   mean/var per token; rstd = exp(-0.5*ln(var+eps)) (stays in the
   ln/exp ACT table set).  Normalization is folded into the PSUM
   evacuations: qk-evac multiplies by rstd broadcast (free dim), v-evac
   multiplies by rstd per-partition scalar.  Mean is subtracted from xT
   once (fp16 2x tensor_tensor).
 - S^T layout [keys, queries]; S matmuls contract over Dh=64, issued as
   64x128 row-tile pairs (partitions 0-63 / 64-127 = two heads) that run
   concurrently in the PE array.
 - softmax normalizer via ones-column appended to V (PV computes
   [O^T | rowsum]); exp without max-subtraction (|S| <= ~6).
 - exp in [128,1024] batches (2 key chunks x 2 heads per ACT call).
"""

import sys

sys.path.insert(0, "/opt/trn_rl_repo")

import numpy as np

import concourse.bacc as bacc
import concourse.bass as bass
import concourse.mybir as mybir
import concourse.tile as tile
from concourse.bass_utils import run_bass_kernel_spmd

B, T, C = 4, 2048, 512
HEADS, DH = 8, 64
NPATCH = 256
EPS = 1e-5
N_CORES = 8
HPC = HEADS // 2          # heads per core = 4
QK_COLS = HPC * DH * 2    # 512 (q block + k block)
V_COLS = HPC * DH         # 256
NT = T // 128             # 16 token tiles
NF = T // NPATCH          # 8 frames
NCC = C // 128            # 4 contraction chunks

F32 = mybir.dt.float32
F32R = mybir.dt.float32r
FP16 = mybir.dt.float16
AF = mybir.ActivationFunctionType
ALU = mybir.AluOpType

_cache = {}
_run_opts = {}      # test harness may set {"trace": True, ...}
_last_res = [None]  # last BassKernelResults, for profiling
_DEBUG = False      # add intermediate-dump outputs to the program


def _build(with_qkv_bias: bool):
    nc = bacc.Bacc("TRN2", target_bir_lowering=False, debug=False,
                   num_devices=N_CORES)
    xt_d = nc.dram_tensor("xt", [C, T], FP16, kind="ExternalInput").ap()
    wqk_d = nc.dram_tensor("wqk", [C, QK_COLS], FP16, kind="ExternalInput").ap()
    wv_d = nc.dram_tensor("wv", [C, V_COLS], FP16, kind="ExternalInput").ap()
    wo_d = nc.dram_tensor("wo", [V_COLS, C], FP16, kind="ExternalInput").ap()
    if with_qkv_bias:
        bqk_d = nc.dram_tensor("bqk", [1, QK_COLS], F32, kind="ExternalInput").ap()
        bv_d = nc.dram_tensor("bv", [1, V_COLS], F32, kind="ExternalInput").ap()
    y_d = nc.dram_tensor("y", [T, C], F32, kind="ExternalOutput").ap()

    with tile.TileContext(nc) as tc:
        _emit(nc, tc, xt_d, wqk_d, wv_d, wo_d, y_d,
              (bqk_d, bv_d) if with_qkv_bias else None)
    nc.compile()
    return nc


def _emit(nc, tc, xt_d, wqk_d, wv_d, wo_d, y_d, biases):
    from contextlib import ExitStack
    ctx = ExitStack()
    with ctx:
        singles = ctx.enter_context(tc.tile_pool(name="singles", bufs=1))
        stp = ctx.enter_context(tc.tile_pool(name="stp", bufs=2))
        ptp = ctx.enter_context(tc.tile_pool(name="ptp", bufs=3))
        recips = ctx.enter_context(tc.tile_pool(name="recips", bufs=3))
        yp = ctx.enter_context(tc.tile_pool(name="yp", bufs=3))
        ps_mm = ctx.enter_context(tc.tile_pool(name="ps_mm", bufs=2, space="PSUM"))
        ps_st = ctx.enter_context(tc.tile_pool(name="ps_st", bufs=2, space="PSUM"))
        ps_pv = ctx.enter_context(tc.tile_pool(name="ps_pv", bufs=2, space="PSUM"))

        # ---- persistent tiles ----
        ones_w = singles.tile([128, 1], FP16)
        nc.vector.memset(ones_w, 1.0)
        eps_t = singles.tile([128, 1], F32)
        nc.vector.memset(eps_t, EPS)

        # weights arrive pre-cast to fp16 from the host (gamma, scale folded)
        wqk = singles.tile([128, NCC, QK_COLS], FP16)
        wv = singles.tile([128, NCC, V_COLS], FP16)
        wo = singles.tile([128, 2, C], FP16)
        nc.sync.dma_start(
            out=wqk, in_=wqk_d.rearrange("(cc p) n -> p cc n", p=128))
        nc.sync.dma_start(
            out=wv, in_=wv_d.rearrange("(cc p) n -> p cc n", p=128))
        nc.sync.dma_start(
            out=wo, in_=wo_d.rearrange("(i p) n -> p i n", p=128))

        if biases is not None:
            bqk_d, bv_d = biases
            bqk_sb = singles.tile([1, QK_COLS], F32)
            nc.gpsimd.dma_start(out=bqk_sb, in_=bqk_d)
            bv_sb = singles.tile([1, V_COLS], F32)
            nc.gpsimd.dma_start(out=bv_sb, in_=bv_d)
            bqk16 = singles.tile([1, QK_COLS], FP16)
            nc.vector.tensor_copy(out=bqk16, in_=bqk_sb)
            bv16 = singles.tile([1, V_COLS], FP16)
            nc.vector.tensor_copy(out=bv16, in_=bv_sb)
            ones_row16 = singles.tile([1, T], FP16)
            nc.vector.memset(ones_row16, 1.0)

        # big persistent activations
        xts = singles.tile([128, NCC, T], FP16)      # raw x^T chunks
        xsq = singles.tile([128, NCC, T], FP16)      # x^T squared
        xct = singles.tile([128, NCC, T], FP16)      # x^T - mu
        xnt = singles.tile([128, NCC, T], FP16)      # (x^T - mu) * rstd
        qkT = singles.tile([128, NCC, T], FP16)      # d0,d1 = q(h01),q(h23); d2,d3 = k
        v_all = singles.tile([128, NT, HPC, DH + 1], FP16)   # V plus ones col
        oT = singles.tile([128, 2, T], FP16)         # [inner dims, tok]

        ones_stage = singles.tile([128, NT * HPC], F32)
        nc.vector.memset(ones_stage, 1.0)
        nc.vector.tensor_copy(
            out=v_all[:, :, :, DH:DH + 1].rearrange("p t h o -> p (t h o)"),
            in_=ones_stage)

        # LayerNorm stat rows (all [1, T], computed piecewise per 512 toks)
        mu_row = singles.tile([1, T], F32)
        m2_row = singles.tile([1, T], F32)
        msq_row = singles.tile([1, T], F32)
        var_row = singles.tile([1, T], F32)
        lnv_row = singles.tile([1, T], F32)
        mu_row16 = singles.tile([1, T], FP16)
        rstd_row16 = singles.tile([1, T], FP16)
        mub = singles.tile([128, T], FP16)
        rstdb = singles.tile([128, T], FP16)

        # ---- stage A: load xT, LayerNorm stats via matmul (piecewise) ----
        for cc in range(NCC):
            nc.sync.dma_start(out=xts[:, cc, :],
                              in_=xt_d[cc * 128:(cc + 1) * 128, :])
        for cc in range(NCC):
            nc.vector.tensor_tensor(
                out=xsq[:, cc, :], in0=xts[:, cc, :], in1=xts[:, cc, :],
                op=ALU.mult)
        for n in range(4):
            sl = slice(n * 512, (n + 1) * 512)
            ms = ps_mm.tile([128, 512], F32, tag="ps_mm")
            for cc in range(NCC):
                nc.tensor.matmul(ms[0:1, :], ones_w, xts[:, cc, sl],
                                 start=(cc == 0), stop=(cc == NCC - 1))
            nc.scalar.activation(out=mu_row[:, sl], in_=ms[0:1, :],
                                 func=AF.Copy, scale=1.0 / C)
            mq = ps_mm.tile([128, 512], F32, tag="ps_mm")
            for cc in range(NCC):
                nc.tensor.matmul(mq[0:1, :], ones_w, xsq[:, cc, sl],
                                 start=(cc == 0), stop=(cc == NCC - 1))
            nc.scalar.activation(out=m2_row[:, sl], in_=mq[0:1, :],
                                 func=AF.Copy, scale=1.0 / C)
            # var = E[x^2] - mu^2 ; rstd = exp(-0.5*ln(var+eps))
            nc.vector.tensor_tensor(out=msq_row[:, sl], in0=mu_row[:, sl],
                                    in1=mu_row[:, sl], op=ALU.mult)
            nc.vector.tensor_tensor(out=var_row[:, sl], in0=m2_row[:, sl],
                                    in1=msq_row[:, sl], op=ALU.subtract)
            nc.scalar.activation(out=lnv_row[:, sl], in_=var_row[:, sl],
                                 func=AF.Ln, bias=eps_t[0:1, :])
            nc.scalar.activation(out=rstd_row16[:, sl], in_=lnv_row[:, sl],
                                 func=AF.Exp, scale=-0.5)
            nc.vector.tensor_copy(out=mu_row16[:, sl], in_=mu_row[:, sl])
            # broadcast along partitions (gpsimd)
            nc.gpsimd.partition_broadcast(mub[:, sl], mu_row16[:, sl])
            nc.gpsimd.partition_broadcast(rstdb[:, sl], rstd_row16[:, sl])
            # xn = (x - mu) * rstd (fp16 2x)
            for cc in range(NCC):
                nc.vector.tensor_tensor(
                    out=xct[:, cc, sl], in0=xts[:, cc, sl], in1=mub[:, sl],
                    op=ALU.subtract)
            for cc in range(NCC):
                nc.vector.tensor_tensor(
                    out=xnt[:, cc, sl], in0=xct[:, cc, sl], in1=rstdb[:, sl],
                    op=ALU.mult)

        if _DEBUG:
            for name, t_ in [("d_mu_row", mu_row), ("d_m2_row", m2_row),
                             ("d_rstd_row16", rstd_row16),
                             ("d_mub", mub), ("d_rstdb", rstdb)]:
                dd = nc.dram_tensor(name, list(t_.shape), t_.dtype,
                                    kind="ExternalOutput").ap()
                nc.sync.dma_start(out=dd, in_=t_[:])

        # ---- stage B: qkT = w_qk^T @ xn^T ; v = xn @ w_v (n-piece major
        # so attention on early frames can start before stage B finishes) ----
        for n in range(4):
            sl = slice(n * 512, (n + 1) * 512)
            for t in range(4 * n, 4 * n + 4):
                mv = ps_mm.tile([128, 512], F32, tag="ps_mm")
                last = NCC - 1 if biases is None else NCC
                for cc in range(NCC):
                    nc.tensor.matmul(
                        mv[:, 0:V_COLS],
                        xnt[:, cc, t * 128:(t + 1) * 128],
                        wv[:, cc, :],
                        start=(cc == 0), stop=(cc == last))
                if biases is not None:
                    nc.tensor.matmul(
                        mv[:, 0:V_COLS],
                        ones_row16[0:1, t * 128:(t + 1) * 128],
                        bv16[0:1, :],
                        start=False, stop=True)
                nc.vector.tensor_copy(
                    out=v_all[:, t, :, 0:DH],
                    in_=mv[:, 0:V_COLS].rearrange("p (h d) -> p h d", h=HPC))
            for d in (0, 2, 1, 3):
                mm = ps_mm.tile([128, 512], F32, tag="ps_mm")
                last = NCC - 1 if biases is None else NCC
                for cc in range(NCC):
                    nc.tensor.matmul(
                        mm, wqk[:, cc, d * 128:(d + 1) * 128],
                        xnt[:, cc, sl],
                        start=(cc == 0), stop=(cc == last))
                if biases is not None:
                    nc.tensor.matmul(
                        mm, bqk16[0:1, d * 128:(d + 1) * 128],
                        ones_row16[0:1, sl],
                        start=False, stop=True)
                nc.vector.tensor_copy(out=qkT[:, d, sl], in_=mm)

        if _DEBUG:
            for name, t_ in [("d_qkT", qkT), ("d_vall", v_all), ("d_xnt", xnt)]:
                dd = nc.dram_tensor(name, list(t_.shape), t_.dtype,
                                    kind="ExternalOutput").ap()
                nc.sync.dma_start(out=dd, in_=t_[:])
            d_oT = nc.dram_tensor("d_oT", [128, 2, T], FP16,
                                  kind="ExternalOutput").ap()

        # ---- stage C: attention.  Frames processed in PAIRS: key chunks
        # visible to both frames stream 512 queries per matmul; per
        # head-pair the two heads run as concurrent 64x128 row tiles.
        # D: out-proj per frame pair. ----
        def finalize(pv, p, f):
            """softmax-normalize pv [65, 512] (h even | h odd) into oT."""
            q_sl = slice(f * 256, (f + 1) * 256)
            ssum = recips.tile([1, 512], F32)
            nc.vector.tensor_copy(out=ssum, in_=pv[DH:DH + 1, :])
            rec = recips.tile([1, 512], F32)
            nc.vector.reciprocal_approx_fast(out=rec, in_=ssum)
            rrep = recips.tile([64, 512], F32)
            nc.gpsimd.partition_broadcast(rrep, rec)
            nc.vector.tensor_tensor(
                out=oT[0:64, p, q_sl],
                in0=pv[0:DH, 0:256], in1=rrep[:, 0:256], op=ALU.mult)
            nc.vector.tensor_tensor(
                out=oT[64:128, p, q_sl],
                in0=pv[0:DH, 256:512], in1=rrep[:, 256:512], op=ALU.mult)

        def pv_mm(pv, kc, h, pt_sl, start, stop):
            # interleaved accumulation chains share a PSUM bank: only the
            # bank's very first matmul may use start=True (it clears the
            # whole bank's has_written bits)
            nc.tensor.matmul(
                pv[0:DH + 1, (h % 2) * 256:(h % 2) * 256 + 256],
                v_all[:, kc, h, :], pt_sl,
                start=start, stop=stop, skip_group_check=True)

        for fp in range(NF // 2):
            f0, f1 = 2 * fp, 2 * fp + 1
            nkc0 = 2 * (f0 + 1)      # key chunks shared by both frames
            q2_sl = slice(f0 * 256, (f1 + 1) * 256)   # 512 queries
            q1_sl = slice(f1 * 256, (f1 + 1) * 256)
            for p in range(2):       # head pair: heads (2p, 2p+1)
                dq, dk = p, 2 + p
                pvA = ps_pv.tile([128, 512], F32, tag="ps_pv")  # frame f0
                pvB = ps_pv.tile([128, 512], F32, tag="ps_pv")  # frame f1
                for kc in range(nkc0):
                    k_sl = slice(kc * 128, (kc + 1) * 128)
                    st = ps_st.tile([128, 1024], F32, tag="ps_st")
                    nc.tensor.matmul(
                        st[:, 0:512],
                        qkT[0:64, dk, k_sl], qkT[0:64, dq, q2_sl],
                        start=True, stop=True)
                    nc.tensor.matmul(
                        st[:, 512:1024],
                        qkT[64:128, dk, k_sl], qkT[64:128, dq, q2_sl],
                        start=True, stop=True)
                    pt = ptp.tile([128, 1024], FP16)
                    nc.scalar.activation(out=pt, in_=st, func=AF.Exp)
                    pv_mm(pvA, kc, 2 * p, pt[:, 0:256],
                          kc == 0, kc == nkc0 - 1)
                    pv_mm(pvA, kc, 2 * p + 1, pt[:, 512:768],
                          False, kc == nkc0 - 1)
                    pv_mm(pvB, kc, 2 * p, pt[:, 256:512], kc == 0, False)
                    pv_mm(pvB, kc, 2 * p + 1, pt[:, 768:1024], False, False)
                finalize(pvA, p, f0)
                # frame f1's two extra key chunks
                st2 = ps_st.tile([128, 1024], F32, tag="ps_st")
                for j in range(2):
                    kc = nkc0 + j
                    k_sl = slice(kc * 128, (kc + 1) * 128)
                    nc.tensor.matmul(
                        st2[:, j * 256:(j + 1) * 256],
                        qkT[0:64, dk, k_sl], qkT[0:64, dq, q1_sl],
                        start=True, stop=True)
                    nc.tensor.matmul(
                        st2[:, 512 + j * 256:512 + (j + 1) * 256],
                        qkT[64:128, dk, k_sl], qkT[64:128, dq, q1_sl],
                        start=True, stop=True)
                pt2 = ptp.tile([128, 1024], FP16)
                nc.scalar.activation(out=pt2, in_=st2, func=AF.Exp)
                for j in range(2):
                    kc = nkc0 + j
                    pv_mm(pvB, kc, 2 * p, pt2[:, j * 256:(j + 1) * 256],
                          False, j == 1)
                    pv_mm(pvB, kc, 2 * p + 1,
                          pt2[:, 512 + j * 256:512 + (j + 1) * 256],
                          False, j == 1)
                finalize(pvB, p, f1)
            if _DEBUG:
                nc.sync.dma_start(out=d_oT[:, :, q2_sl], in_=oT[:, :, q2_sl])
            # out-projection for this frame pair's four token tiles
            for t in range(4 * fp, 4 * fp + 4):
                ym = ps_mm.tile([128, 512], F32, tag="ps_mm")
                for i in range(2):
                    nc.tensor.matmul(
                        ym, oT[:, i, t * 128:(t + 1) * 128], wo[:, i, :],
                        start=(i == 0), stop=(i == 1))
                ysb = yp.tile([128, C], F32)
                nc.vector.tensor_copy(out=ysb, in_=ym)
                nc.gpsimd.dma_start(
                    out=y_d[t * 128:(t + 1) * 128, :], in_=ysb)


def kernel(x, ln_gamma, ln_beta, w_qkv, w_out, b_out, mask):
    x = np.asarray(x, dtype=np.float32)
    ln_gamma = np.asarray(ln_gamma, dtype=np.float32)
    ln_beta = np.asarray(ln_beta, dtype=np.float32)
    w_qkv = np.asarray(w_qkv, dtype=np.float32)
    w_out = np.asarray(w_out, dtype=np.float32)
    b_out = np.asarray(b_out, dtype=np.float32)

    inner = HEADS * DH
    wq_all = w_qkv[:, 0:inner] * ln_gamma[:, None]
    wk_all = w_qkv[:, inner:2 * inner] * ln_gamma[:, None]
    wv_all = w_qkv[:, 2 * inner:3 * inner] * ln_gamma[:, None]
    scale = DH ** -0.5
    # beta contribution to q/k/v (exact: qkv = ln(x)@(gamma*W) + beta@W)
    bq_all = ln_beta @ w_qkv[:, 0:inner]
    bk_all = ln_beta @ w_qkv[:, inner:2 * inner]
    bv_all = ln_beta @ w_qkv[:, 2 * inner:3 * inner]
    with_bias = bool(
        np.abs(bq_all).max() > 0 or np.abs(bk_all).max() > 0
        or np.abs(bv_all).max() > 0)

    key = ("prog", with_bias)
    if key not in _cache:
        _cache[key] = _build(with_bias)
    nc = _cache[key]

    in_maps = []
    for c in range(N_CORES):
        b = c // 2
        h0 = (c % 2) * HPC
        cols = slice(h0 * DH, (h0 + HPC) * DH)
        wqk_c = np.concatenate([wq_all[:, cols] * scale, wk_all[:, cols]],
                               axis=1)
        m = {
            "xt": np.ascontiguousarray(x[b].T.astype(np.float16)),
            "wqk": np.ascontiguousarray(wqk_c.astype(np.float16)),
            "wv": np.ascontiguousarray(wv_all[:, cols].astype(np.float16)),
            "wo": np.ascontiguousarray(w_out[cols, :].astype(np.float16)),
        }
        if with_bias:
            bqk_c = np.concatenate([bq_all[cols] * scale, bk_all[cols]])
            m["bqk"] = np.ascontiguousarray(bqk_c[None, :].astype(np.float32))
            m["bv"] = np.ascontiguousarray(bv_all[cols][None, :].astype(np.float32))
        in_maps.append(m)

    res = run_bass_kernel_spmd(nc, in_maps, core_ids=list(range(N_CORES)),
                               **_run_opts)
    _last_res[0] = res
    y = np.empty((B, T, C), dtype=np.float32)
    for b in range(B):
        y[b] = res.results[2 * b]["y"] + res.results[2 * b + 1]["y"] + b_out
    return y
